# revision 18
# baseline (speedup 1.0000x reference)
"""Trainium2 Bass kernel for the GPT2Shared multimodal ensemble MLP.

Pipeline (per token): three modality adapters (Linear+GELU) -> shared
projection -> concat -> 32-expert ensemble MLP (2304->768->32->5, relu) ->
mean over experts -> mean over time.

Sharding: pure data-parallel over the batch dim. Each of the 8 cores gets
4 batches (1024 tokens) and runs the whole pipeline for its tokens; the
final reduction over experts+time happens on-device, so each core emits a
[5, 4] partial and the host only rescales/concats.

All on-device tensors live in [feature, token] layout so every matmul uses
the natural weight layout as the stationary operand and no transposes are
needed anywhere. The heavy GEMMs (adapters, shared projection, ensemble
layer 1/2) run in fp8e4m3 with DoubleRow perf mode (2 MACs/cell/cycle,
K=256 per instruction), which halves tensor-engine cycles vs bf16.
Weights are pre-scaled x32 on host so they sit in fp8's sweet spot; the
following activation's scale=1/32 compensates exactly. The tiny final
head (K=1024, M=5) stays bf16.
"""

import os
import sys

for _p in ("/opt/trn_rl_repo", "/root/.axon_site/_ro/trn_rl_repo"):
    if os.path.isdir(_p) and _p not in sys.path:
        sys.path.append(_p)

import ml_dtypes
import numpy as np

import concourse.bass as bass
import concourse.tile as tile
from concourse import bacc, mybir
from concourse.bass_utils import run_bass_kernel_spmd

BF16 = mybir.dt.bfloat16
FP8 = mybir.dt.float8e4
F32 = mybir.dt.float32
NPBF = ml_dtypes.bfloat16
NPF8 = ml_dtypes.float8_e4m3  # TRN FP8_EXP4: max +-240
DR = mybir.MatmulPerfMode.DoubleRow

N_CORES = 8
B, T = 32, 256
TOK = B * T // N_CORES          # 1024 tokens per core
BPC = B // N_CORES              # 4 batches per core
NT, NSZ = 2, 512                # token tiles per core
GKT = 18                        # 2304 gelu/chunk features = 18 k-tiles
GKP = 9                         # ... = 9 DoubleRow k-pairs
PKT = 6                         # 768 features = 6 k-tiles
PKP = 3                         # ... = 3 DoubleRow k-pairs
E, HID, TGT = 32, 32, 5
WS = 32.0                       # fp8 weight pre-scale
SI = float(1.0 / WS)
# (name, in_dim, in_ktiles, chunk row-tile offset) in reference concat order
# chunk = [video, text, audio]
MODS = (("v", 768, 6, 0), ("t", 768, 6, 6), ("a", 1024, 8, 12))

_NC = None
LAST_RESULT = None


def _build():
    nc = bacc.Bacc("TRN2", target_bir_lowering=False, debug=False,
                   num_devices=N_CORES)

    dr = {}
    for mn, kin, kint, _ in MODS:
        dr[f"x{mn}"] = nc.dram_tensor(f"x{mn}", [128, kint, TOK], FP8, kind="ExternalInput")
        dr[f"W{mn}"] = nc.dram_tensor(f"W{mn}", [128, kint, 2304], FP8, kind="ExternalInput")
        dr[f"b{mn}"] = nc.dram_tensor(f"b{mn}", [128, GKT], F32, kind="ExternalInput")
    dr["Wp"] = nc.dram_tensor("Wp", [128, GKT, 768], FP8, kind="ExternalInput")
    dr["bp"] = nc.dram_tensor("bp", [128, PKT], F32, kind="ExternalInput")
    # host-rearranged: [p, e, kt, m] <- We1[e, kt*128+p, m]
    dr["We1"] = nc.dram_tensor("We1", [128, E, GKT, 768], FP8, kind="ExternalInput")
    dr["be1"] = nc.dram_tensor("be1", [128, E, PKT], F32, kind="ExternalInput")
    # host-rearranged: [p, kt, e, h] <- We2[e, kt*128+p, h]
    dr["We2"] = nc.dram_tensor("We2", [128, PKT, E, HID], BF16, kind="ExternalInput")
    # col-tiling packed: be2p[q*32+h, g] = be2[4g+q, h]
    dr["be2"] = nc.dram_tensor("be2", [128, E // 4], F32, kind="ExternalInput")
    # host-stacked: [p, kt, t] <- We3[(kt*128+p)//32, (kt*128+p)%32, t]
    dr["We3"] = nc.dram_tensor("We3", [128, 8, TGT], BF16, kind="ExternalInput")
    out_d = nc.dram_tensor("out", [TGT, BPC], F32, kind="ExternalOutput")

    gelu = mybir.ActivationFunctionType.Gelu_apprx_tanh
    relu = mybir.ActivationFunctionType.Relu
    ident = mybir.ActivationFunctionType.Identity

    with tile.TileContext(nc) as tc:
        with (
            tc.tile_pool(name="const", bufs=1) as constp,
            tc.tile_pool(name="persist", bufs=1) as perp,
            tc.tile_pool(name="psA", bufs=4, space=bass.MemorySpace.PSUM) as psA,
            # created before the adapter pools so the first We1 DMAs can
            # prefetch while the adapters compute
            tc.tile_pool(name="we1p", bufs=2) as we1p,
        ):
            # small constants — on the gpsimd DMA queue so they stay off the
            # critical path to the first matmul (sync queue)
            be1_sb = constp.tile([128, E, PKT], F32, tag="be1")
            nc.gpsimd.dma_start(be1_sb[:], dr["be1"][:])
            we2_sb = constp.tile([128, PKT, E, HID], BF16, tag="we2")
            nc.gpsimd.dma_start(we2_sb[:], dr["We2"][:])
            be2_sb = constp.tile([128, E // 4], F32, tag="be2")
            nc.gpsimd.dma_start(be2_sb[:], dr["be2"][:])
            we3_sb = constp.tile([128, 8, TGT], BF16, tag="we3")
            nc.gpsimd.dma_start(we3_sb[:], dr["We3"][:])
            bp_sb = constp.tile([128, PKT], F32, tag="bp")
            nc.gpsimd.dma_start(bp_sb[:], dr["bp"][:])

            chunk_sb = perp.tile([128, GKT, TOK], FP8, tag="chunk")

            # ---------------- adapters + shared projection ----------------
            with (
                tc.tile_pool(name="adw", bufs=1) as adw,
                tc.tile_pool(name="adf", bufs=2) as adf,
            ):
                # DMA order is the critical path to the first matmul: the
                # first modality's weights/features go first, in chunks so
                # the gelu chains can start after ~1MB instead of ~4MB; Wp
                # and the features ride separate DMA queues.
                wp_sb = adw.tile([128, GKT, 768], FP8, tag="wp")
                for mi, (mn, kin, kint, coff) in enumerate(MODS):
                    bm_sb = constp.tile([128, GKT], F32, tag=f"b{mn}")
                    nc.sync.dma_start(bm_sb[:], dr[f"b{mn}"][:])
                    wm_sb = adw.tile([128, 8, 2304], FP8, tag="wmod")
                    for c in range(3):
                        nc.sync.dma_start(
                            wm_sb[:, :kint, 768 * c:768 * (c + 1)],
                            dr[f"W{mn}"][:, :, 768 * c:768 * (c + 1)])
                    if mi == 0:
                        nc.sync.dma_start(wp_sb[:], dr["Wp"][:])
                    f_sb = adf.tile([128, 8, TOK], FP8, tag="feat")
                    for h in range(NT):
                        nc.scalar.dma_start(
                            f_sb[:, :kint, h * NSZ:(h + 1) * NSZ],
                            dr[f"x{mn}"][:, :, h * NSZ:(h + 1) * NSZ])
                    g_sb = adw.tile([128, GKT, TOK], FP8, tag="g")
                    # g = gelu(x @ Wm + bm), in [feature, token] layout
                    for n in range(NT):
                        for gf in range(GKT):
                            ps = psA.tile([128, NSZ], F32, tag="ps")
                            for kp in range(kint // 2):
                                nc.tensor.matmul(
                                    ps[:],
                                    wm_sb[:, 2 * kp:2 * kp + 2,
                                          gf * 128:(gf + 1) * 128],
                                    f_sb[:, 2 * kp:2 * kp + 2,
                                         n * NSZ:(n + 1) * NSZ],
                                    start=(kp == 0), stop=(kp == kint // 2 - 1),
                                    perf_mode=DR)
                            nc.scalar.activation(
                                g_sb[:, gf, n * NSZ:(n + 1) * NSZ], ps[:],
                                gelu, bias=bm_sb[:, gf:gf + 1], scale=SI)
                    # chunk rows [coff:coff+6] = g @ Wp + bp
                    for n in range(NT):
                        for pf in range(PKT):
                            ps = psA.tile([128, NSZ], F32, tag="ps")
                            for kp in range(GKP):
                                nc.tensor.matmul(
                                    ps[:],
                                    wp_sb[:, 2 * kp:2 * kp + 2,
                                          pf * 128:(pf + 1) * 128],
                                    g_sb[:, 2 * kp:2 * kp + 2,
                                         n * NSZ:(n + 1) * NSZ],
                                    start=(kp == 0), stop=(kp == GKP - 1),
                                    perf_mode=DR)
                            nc.scalar.activation(
                                chunk_sb[:, coff + pf, n * NSZ:(n + 1) * NSZ],
                                ps[:], ident, bias=bp_sb[:, pf:pf + 1], scale=SI)

            # ---------------- ensemble ----------------
            h2_sb = perp.tile([128, 8, TOK], BF16, tag="h2")
            with (
                tc.tile_pool(name="h1p", bufs=8) as h1p,
                tc.tile_pool(name="psB", bufs=2, space=bass.MemorySpace.PSUM) as psB,
            ):
                # experts in groups of 4: layer 1 per expert, then layer 2
                # with 4 experts col-tiled into the 128-wide PE array
                for g in range(E // 4):
                    h1g = []
                    for q in range(4):
                        e = 4 * g + q
                        w1_sb = we1p.tile([128, GKT, 768], FP8, tag="w1")
                        nc.gpsimd.dma_start(w1_sb[:], dr["We1"][:, e])
                        h1_sb = h1p.tile([128, PKT, TOK], BF16, tag="h1")
                        h1g.append(h1_sb)
                        for n in range(NT):
                            for pf in range(PKT):
                                ps = psA.tile([128, NSZ], F32, tag="ps")
                                for kp in range(GKP):
                                    nc.tensor.matmul(
                                        ps[:],
                                        w1_sb[:, 2 * kp:2 * kp + 2,
                                              pf * 128:(pf + 1) * 128],
                                        chunk_sb[:, 2 * kp:2 * kp + 2,
                                                 n * NSZ:(n + 1) * NSZ],
                                        start=(kp == 0), stop=(kp == GKP - 1),
                                        perf_mode=DR)
                                nc.scalar.activation(
                                    h1_sb[:, pf, n * NSZ:(n + 1) * NSZ], ps[:],
                                    relu, bias=be1_sb[:, e, pf:pf + 1], scale=SI)
                    for n in range(NT):
                        ps2 = psB.tile([128, NSZ], F32, tag="ps2")
                        for kt in range(PKT):
                            for q in range(4):
                                nc.tensor.matmul(
                                    ps2[q * 32:(q + 1) * 32, :],
                                    we2_sb[:, kt, 4 * g + q, :],
                                    h1g[q][:, kt, n * NSZ:(n + 1) * NSZ],
                                    start=(kt == 0), stop=(kt == PKT - 1),
                                    tile_position=(0, q * 32))
                        nc.scalar.activation(
                            h2_sb[:, g, n * NSZ:(n + 1) * NSZ],
                            ps2[:], relu, bias=be2_sb[:, g:g + 1])

                # ensemble head: accumulate all 32 experts' 5-dim outputs and
                # reduce over time within each batch
                s_sb = constp.tile([TGT, BPC], F32, tag="s")
                for n in range(NT):
                    ps3 = psB.tile([TGT, NSZ], F32, tag="ps3")
                    for kt in range(8):
                        nc.tensor.matmul(
                            ps3[:],
                            we3_sb[:, kt, :],
                            h2_sb[:, kt, n * NSZ:(n + 1) * NSZ],
                            start=(kt == 0), stop=(kt == 7))
                    nc.vector.reduce_sum(
                        s_sb[:, 2 * n:2 * n + 2],
                        ps3[:].rearrange("p (g t) -> p g t", t=T),
                        axis=mybir.AxisListType.X)
                nc.sync.dma_start(out_d[:], s_sb[:])

    nc.compile()
    return nc


def _f8w(x):
    """Scale x32, clip to TRN e4m3 range, cast to fp8."""
    return np.clip(np.asarray(x, np.float32) * WS, -240.0, 240.0).astype(NPF8)


def _f8(x):
    return np.clip(np.asarray(x, np.float32), -240.0, 240.0).astype(NPF8)


def _prep(inputs):
    """Host-side: cast to fp8/bf16, pre-rearrange to the on-device layouts,
    build per-core input maps."""
    f32 = np.float32

    # features: [B*T, F] -> [128, kt, N] fp8 (p-major per k-tile)
    featsP = {}
    for mn, kin, kint, _ in MODS:
        key = {"v": "video_feat", "t": "text_feat", "a": "audio_feat"}[mn]
        x = np.asarray(inputs[key], f32).reshape(B * T, kin)
        featsP[mn] = np.ascontiguousarray(
            _f8(x).T.reshape(kint, 128, B * T).transpose(1, 0, 2))

    shared = {}
    wkeys = {"v": "Wv", "t": "Wt", "a": "Wa"}
    bkeys = {"v": "bv", "t": "bt", "a": "ba"}
    for mn, kin, kint, _ in MODS:
        shared[f"W{mn}"] = np.ascontiguousarray(
            _f8w(inputs[wkeys[mn]]).reshape(kint, 128, 2304).transpose(1, 0, 2))
        shared[f"b{mn}"] = np.ascontiguousarray(
            np.asarray(inputs[bkeys[mn]], f32).reshape(GKT, 128).T)
    shared["Wp"] = np.ascontiguousarray(
        _f8w(inputs["Wp"]).reshape(GKT, 128, 768).transpose(1, 0, 2))
    shared["bp"] = np.ascontiguousarray(
        np.asarray(inputs["bp"], f32).reshape(PKT, 128).T)
    shared["We1"] = np.ascontiguousarray(
        _f8w(inputs["We1"]).reshape(E, GKT, 128, 768).transpose(2, 0, 1, 3))
    shared["be1"] = np.ascontiguousarray(
        np.asarray(inputs["be1"], f32).reshape(E, PKT, 128).transpose(2, 0, 1))
    shared["We2"] = np.ascontiguousarray(
        np.asarray(inputs["We2"], np.float32).astype(NPBF)
        .reshape(E, PKT, 128, HID).transpose(2, 1, 0, 3))
    shared["be2"] = np.ascontiguousarray(
        np.asarray(inputs["be2"], f32).reshape(E // 4, 4, HID)
        .transpose(1, 2, 0).reshape(128, E // 4))
    shared["We3"] = np.ascontiguousarray(
        np.asarray(inputs["We3"], f32).astype(NPBF)
        .reshape(8, 128, TGT).transpose(1, 0, 2))

    in_maps = []
    for c in range(N_CORES):
        m = dict(shared)
        sl = slice(c * TOK, (c + 1) * TOK)
        for mn, _, _, _ in MODS:
            m[f"x{mn}"] = np.ascontiguousarray(featsP[mn][:, :, sl])
        in_maps.append(m)
    be3_sum = np.asarray(inputs["be3"], f32).sum(axis=0)
    return in_maps, be3_sum


def kernel(**inputs):
    global _NC, LAST_RESULT
    if _NC is None:
        _NC = _build()
    in_maps, be3_sum = _prep(inputs)
    trace = bool(os.environ.get("BASS_KERNEL_TRACE"))
    kwargs = {}
    if trace:
        import concourse.bass_utils as _bu
        _bu.upload_artifacts = lambda d: d  # no artifact bucket here
        kwargs["tmpdir"] = os.environ.get("BASS_KERNEL_TRACE_DIR") or None
    res = run_bass_kernel_spmd(_NC, in_maps, list(range(N_CORES)),
                               trace=trace, **kwargs)
    LAST_RESULT = res
    logits = np.empty((B, TGT), np.float32)
    for c in range(N_CORES):
        s = res.results[c]["out"]  # [TGT, BPC]
        logits[c * BPC:(c + 1) * BPC] = ((s + be3_sum[:, None] * T) / (E * T)).T
    return logits
